# revision 31
# baseline (speedup 1.0000x reference)
"""Trainium2 Bass kernel for nn_ExpEncoder (pooling).

Computation (reference):
  E = emb_gene[omc_idx]                                  [B, G, D]
  proj = E @ w0 + b0                                     [B, G, A]
  ctx = emb_ptw[ptw_ids[0]]                              [P, A]
  t = tanh(proj[:,None] + ctx[None,:,None])              [B, P, G, A]
  logits = t @ beta_w + beta_b                           [B, P, G, H]
  attn = softmax(logits, axis=2); w = attn.sum(-1)       [B, P, G]
  out = einsum('bpg,bgd->bpd', w, E)                     [B, P, D]

Sharding: data-parallel over B across 8 cores (2 batches/core), params
replicated.  The kernel is ACT(tanh)-bound; layout puts (p2, a64) pairs on
SBUF partitions so the ctx broadcast-add runs as DVE tensor_scalar
(per-partition scalar) in bf16 4x mode and the beta contraction runs as
block-diagonal bf16 matmuls straight into a [8*p + h, g] PSUM layout.
"""

import os
import sys

for _p in ("/opt/trn_rl_repo", os.path.expanduser("~/.axon_site/_ro/trn_rl_repo")):
    if os.path.isdir(_p) and _p not in sys.path:
        sys.path.insert(0, _p)

from contextlib import ExitStack

import ml_dtypes
import numpy as np

import concourse.bass as bass
import concourse.mybir as mybir
import concourse.tile as tile
from concourse import bacc
from concourse.bass_utils import run_bass_kernel_spmd

F32 = mybir.dt.float32
BF16 = mybir.dt.bfloat16
I32 = mybir.dt.int32
NPBF16 = np.dtype(ml_dtypes.bfloat16)

B, P, G = 16, 32, 512
D, A, H = 512, 400, 8
OMC1, PTW = 20001, 1000
NCORES = 8
BLOC = B // NCORES          # batches per core = 2
NC_MAIN = 6                 # main a-chunks of 64 (a in [64c, 64c+64))
AREM = 16                   # remainder a in [384, 400)
NPH = 2                     # p-halves (units per batch), 16 p's each
NPG = 8                     # p-groups of 2 within a unit
NT = 2                      # logits psum tiles per unit (4 p-groups each)


def _emit(ctx, tc, t_ap):
    """Emit the whole per-core program under TileContext tc.

    t_ap: dict of DRAM APs by name.
    """
    nc = tc.nc
    emb = t_ap["emb_gene"]
    idx = t_ap["idx_loc"]
    out_d = t_ap["out_loc"]

    const = ctx.enter_context(tc.tile_pool(name="const", bufs=1))

    # ---- load constants / aux inputs ------------------------------------
    idx_sb = const.tile([128, 8], I32)
    nc.sync.dma_start(out=idx_sb[:, :], in_=idx.rearrange("(j p) -> p j", p=128))

    w0rep_sb = const.tile([128, 4 * NC_MAIN * 128], BF16)   # (k, c, (p2,a64))
    nc.sync.dma_start(
        out=w0rep_sb[:, :].rearrange("p (k m) -> p k m", k=4),
        in_=t_ap["w0_rep"].rearrange("(k p) m -> p k m", p=128),
    )
    w0rem_sb = const.tile([128, 4 * 128], BF16)             # (k, (q4,p2,a16))
    nc.sync.dma_start(
        out=w0rem_sb[:, :].rearrange("p (k m) -> p k m", k=4),
        in_=t_ap["w0_rem"].rearrange("(k p) m -> p k m", p=128),
    )
    ctxsc_sb = const.tile([128, NC_MAIN * NPH * NPG], F32)
    nc.sync.dma_start(out=ctxsc_sb[:, :], in_=t_ap["ctx_sc"][:, :])
    ctxrem_sb = const.tile([128, NPH * NT], F32)
    nc.sync.dma_start(out=ctxrem_sb[:, :], in_=t_ap["ctx_rem"][:, :])
    betabd_sb = const.tile([128, NC_MAIN * 16], BF16)
    nc.sync.dma_start(out=betabd_sb[:, :], in_=t_ap["beta_bd"][:, :])
    betarem_sb = const.tile([128, 128], BF16)
    nc.sync.dma_start(out=betarem_sb[:, :], in_=t_ap["beta_rem"][:, :])
    hsum_sb = const.tile([128, NT * 16], BF16)
    nc.sync.dma_start(out=hsum_sb[:, :], in_=t_ap["hsum"][:, :])
    ident_sb = const.tile([128, 128], F32)
    nc.sync.dma_start(out=ident_sb[:, :], in_=t_ap["ident"][:, :])
    ebias_sb = const.tile([128, 1], F32)
    nc.sync.dma_start(out=ebias_sb[:, :], in_=t_ap["exp_bias"][:, :])
    # dummy tanh: hoists the ACT_TABLE_LOAD (exp_and_others: tanh+exp) into
    # the idle prologue window instead of stalling the first real tanh
    scratch_sb = const.tile([128, 1], F32)
    nc.scalar.activation(
        scratch_sb[:, :], ebias_sb[:, :], mybir.ActivationFunctionType.Tanh
    )

    # ---- gather gene embeddings (8 tiles of 128 rows, inside prologue_b) -
    E_sb = const.tile([128, 8 * D], F32)          # tile j cols [j*512, +512)

    ET_sb = const.tile([128, 4 * 1024], BF16)     # chunk k cols [k*1024 + bg]
    projT_sb = const.tile([128, BLOC * NC_MAIN * G], BF16)  # (b, c) -> [128,512]
    remT_sb = const.tile([128, BLOC * G], BF16)             # (b) -> [128,512]
    wT_sb = const.tile([128, BLOC * 128], F32)              # (b, gc*32 + p)

    # logits psum tiles allocated + zeroed up-front (garbage rows must stay 0)
    lpsum = ctx.enter_context(tc.tile_pool(name="lpsum", bufs=1, space="PSUM"))
    lp_tiles = []
    for i in range(4):
        lp = lpsum.tile([128, G], F32, tag=f"lp{i}", name=f"lp{i}")
        nc.vector.memset(lp[:, :], 0.0)
        lp_tiles.append(lp)

    ppsum = ctx.enter_context(tc.tile_pool(name="ppsum", bufs=1, space="PSUM"))

    def prologue_b(b, et_on_act):
        """gather + E^T transposes + proj for one batch.

        One multi-row indirect gather per batch; rem proj chunk first (the
        unit's first tanh is the rem tile, so it gates the ACT stream).
        """
        for j0 in range(4 * b, 4 * b + 4):
            nc.gpsimd.indirect_dma_start(
                out=E_sb[:, j0 * D:(j0 + 1) * D],
                out_offset=None,
                in_=emb[:, :],
                in_offset=bass.IndirectOffsetOnAxis(
                    ap=idx_sb[:, j0:j0 + 1], axis=0),
            )
        for j in range(4 * b, 4 * b + 4):
            for k in range(4):
                tp = ppsum.tile([128, 128], F32, tag="tp", name="tp", bufs=2)
                nc.tensor.transpose(
                    out=tp[:, :],
                    in_=E_sb[:, j * D + k * 128: j * D + (k + 1) * 128],
                    identity=ident_sb[:, :],
                )
                # split PSUM->SBUF copies across ACT and DVE in the b=0
                # prologue (both idle); b=1: all DVE (ACT is the bottleneck)
                eng = (nc.scalar.copy if (et_on_act and k >= 2)
                       else nc.vector.tensor_copy)
                eng(
                    ET_sb[:, k * 1024 + j * 128: k * 1024 + (j + 1) * 128],
                    tp[:, :],
                )
        # remainder chunk first, (q4,p2,a16) rows
        pr = ppsum.tile([128, G], F32, tag="pp", name="pp")
        for k in range(4):
            nc.tensor.matmul(
                out=pr[:, :],
                lhsT=w0rem_sb[:, k * 128:(k + 1) * 128],
                rhs=ET_sb[:, k * 1024 + b * G: k * 1024 + (b + 1) * G],
                start=(k == 0),
                stop=(k == 3),
            )
        nc.vector.tensor_copy(remT_sb[:, b * G:(b + 1) * G], pr[:, :])
        for c in range(NC_MAIN):
            pp = ppsum.tile([128, G], F32, tag="pp", name="pp")
            for k in range(4):
                nc.tensor.matmul(
                    out=pp[:, :],
                    lhsT=w0rep_sb[:, (k * NC_MAIN + c) * 128:(k * NC_MAIN + c + 1) * 128],
                    rhs=ET_sb[:, k * 1024 + b * G: k * 1024 + (b + 1) * G],
                    start=(k == 0),
                    stop=(k == 3),
                )
            nc.vector.tensor_copy(
                projT_sb[:, (b * NC_MAIN + c) * G:(b * NC_MAIN + c + 1) * G],
                pp[:, :],
            )

    spool = ctx.enter_context(tc.tile_pool(name="spool", bufs=2))
    apool = ctx.enter_context(tc.tile_pool(name="apool", bufs=2))
    wpsum = ctx.enter_context(tc.tile_pool(name="wpsum", bufs=1, space="PSUM"))

    def tanh_stage(b, ph):
        if True:
            # -- broadcast-add + tanh (rem first: its matmul lands early) --
            s_rem = []
            for T in range(NT):
                sr = spool.tile([128, G], BF16, tag=f"sr{T}", name=f"sr{T}")
                nc.scalar.activation(
                    sr[:, :], remT_sb[:, b * G:(b + 1) * G],
                    mybir.ActivationFunctionType.Tanh,
                    bias=ctxrem_sb[:, ph * NT + T: ph * NT + T + 1],
                )
                s_rem.append(sr)
            s_main = []
            for c in range(NC_MAIN):
                s = spool.tile([128, NPG * G], BF16, tag=f"s{c}", name=f"s{c}")
                for pg in range(NPG):
                    nc.vector.tensor_scalar_add(
                        s[:, pg * G:(pg + 1) * G],
                        projT_sb[:, (b * NC_MAIN + c) * G:(b * NC_MAIN + c + 1) * G],
                        ctxsc_sb[:, (c * NPH + ph) * NPG + pg:
                                 (c * NPH + ph) * NPG + pg + 1],
                    )
                nc.scalar.activation(
                    s[:, :], s[:, :], mybir.ActivationFunctionType.Tanh
                )
                s_main.append(s)
            return s_main, s_rem

    def epilogue_a(b, ph, s_main, s_rem):
        u = b * NPH + ph
        if True:
            # -- logits: block-diag beta matmuls into [32*qq + 8*p2 + h] ---
            # c-outer so each matmul level only needs tanh chunk c (matmuls
            # on one psum tile serialize in emission order); rem right after
            # the start=True level so the last level is c=NC_MAIN-1.
            for T in range(NT):
                lp = lp_tiles[(u % 2) * 2 + T]
                for c in range(NC_MAIN):
                    for qq in range(4):
                        pg = T * 4 + qq
                        nc.tensor.matmul(
                            out=lp[32 * qq: 32 * qq + 16, :],
                            lhsT=betabd_sb[:, c * 16:(c + 1) * 16],
                            rhs=s_main[c][:, pg * G:(pg + 1) * G],
                            start=(c == 0),
                            stop=(c == NC_MAIN - 1),
                            skip_group_check=True,
                            tile_position=(0, 32 * qq),
                        )
                    if c == 0:
                        # remainder: M=128, zero cols on unused rows (adds 0)
                        nc.tensor.matmul(
                            out=lp[:, :],
                            lhsT=betarem_sb[:, :],
                            rhs=s_rem[T][:, :],
                            start=False,
                            stop=(NC_MAIN == 1),
                            skip_group_check=True,
                        )

            # -- exp over g (fused beta_b bias + row-sum accumulator) ------
            attns, ssums = [], []
            for T in range(NT):
                lp = lp_tiles[(u % 2) * 2 + T]
                attn = apool.tile([128, G], BF16, tag=f"at{T}", name=f"at{T}")
                ssum = apool.tile([128, 1], F32, tag=f"ss{T}", name=f"ss{T}")
                nc.scalar.activation(
                    attn[:, :], lp[:, :], mybir.ActivationFunctionType.Exp,
                    bias=ebias_sb[:, :], accum_out=ssum[:, :],
                )
                attns.append(attn)
                ssums.append(ssum)
            return attns, ssums

    def epilogue_b(b, ph, attns, ssums):
        if True:
            # -- normalize + head-sum --------------------------------------
            wps = wpsum.tile([16, G], F32, tag="w", name="wps")
            for T in range(NT):
                rinv = apool.tile([128, 1], F32, tag=f"ri{T}", name=f"ri{T}")
                nc.vector.reciprocal(rinv[:, :], ssums[T][:, :])
                ascl = apool.tile([128, G], BF16, tag=f"as{T}", name=f"as{T}")
                nc.vector.tensor_scalar_mul(ascl[:, :], attns[T][:, :], rinv[:, :])
                nc.tensor.matmul(
                    out=wps[:, :],
                    lhsT=hsum_sb[:, T * 16:(T + 1) * 16],
                    rhs=ascl[:, :],
                    start=(T == 0),
                    stop=(T == 1),
                )

            # -- w^T via PE transpose --------------------------------------
            w_sb = apool.tile([16, G], F32, tag="wsb", name="wsb")
            nc.vector.tensor_copy(w_sb[:, :], wps[:, :])
            for gc in range(4):
                wtp = ppsum.tile([128, 16], F32, tag="tp", name="wtp", bufs=2, padded_shape=[128, 128])
                nc.tensor.transpose(
                    out=wtp[:, :],
                    in_=w_sb[:, gc * 128:(gc + 1) * 128],
                    identity=ident_sb[:16, :16],
                )
                nc.vector.tensor_copy(
                    wT_sb[:, b * 128 + gc * 32 + ph * 16:
                          b * 128 + gc * 32 + ph * 16 + 16],
                    wtp[:, :],
                )

    def final_b(b):
        # -- final fp32 matmul: out[b] = w^T.T @ E -------------------------
        ops = wpsum.tile([P, D], F32, tag="w", name="ops")
        for gc in range(4):
            nc.tensor.matmul(
                out=ops[0:P, :],
                lhsT=wT_sb[:, b * 128 + gc * 32: b * 128 + (gc + 1) * 32],
                rhs=E_sb[:, (b * 4 + gc) * D:(b * 4 + gc + 1) * D],
                start=(gc == 0),
                stop=(gc == 3),
            )
        out_sb = apool.tile([P, D], F32, tag="osb", name="out_sb")
        nc.vector.tensor_copy(out_sb[:, :], ops[0:P, :])
        nc.sync.dma_start(out=out_d[b], in_=out_sb[:, :])

    # software-pipelined emission: tanh stages run ahead so no engine's
    # in-order queue blocks the tanh stream (ACT) or the adds (DVE).
    prologue_b(0, et_on_act=True)
    ts00 = tanh_stage(0, 0)
    ts01 = tanh_stage(0, 1)
    ea00 = epilogue_a(0, 0, *ts00)
    prologue_b(1, et_on_act=False)
    ts10 = tanh_stage(1, 0)
    ea01 = epilogue_a(0, 1, *ts01)
    epilogue_b(0, 0, *ea00)
    ts11 = tanh_stage(1, 1)
    ea10 = epilogue_a(1, 0, *ts10)
    epilogue_b(0, 1, *ea01)
    final_b(0)
    ea11 = epilogue_a(1, 1, *ts11)
    epilogue_b(1, 0, *ea10)
    epilogue_b(1, 1, *ea11)
    final_b(1)


def build_program():
    nc = bacc.Bacc(
        "TRN2", target_bir_lowering=False, debug=False, num_devices=NCORES
    )
    t_ap = {}

    def din(name, shape, dt):
        t_ap[name] = nc.dram_tensor(name, shape, dt, kind="ExternalInput").ap()

    din("emb_gene", [OMC1, D], F32)
    din("idx_loc", [BLOC * G], I32)
    din("w0_rep", [D, NC_MAIN * 128], BF16)
    din("w0_rem", [D, 128], BF16)
    din("ctx_sc", [128, NC_MAIN * NPH * NPG], F32)
    din("ctx_rem", [128, NPH * NT], F32)
    din("beta_bd", [128, NC_MAIN * 16], BF16)
    din("beta_rem", [128, 128], BF16)
    din("hsum", [128, NT * 16], BF16)
    din("ident", [128, 128], F32)
    din("exp_bias", [128, 1], F32)
    t_ap["out_loc"] = nc.dram_tensor(
        "out_loc", [BLOC, P, D], F32, kind="ExternalOutput"
    ).ap()

    with tile.TileContext(nc) as tc, ExitStack() as ctx:
        _emit(ctx, tc, t_ap)
    nc.compile()
    return nc


def build_aux(ptw_ids, emb_ptw, w0, b0, beta_w, beta_b):
    """Host-side constant tensors (shared across cores)."""
    ptw_ids = np.asarray(ptw_ids).astype(np.int64)
    emb_ptw = np.asarray(emb_ptw, dtype=np.float32)
    w0 = np.asarray(w0, dtype=np.float32)
    b0 = np.asarray(b0, dtype=np.float32)
    beta_w = np.asarray(beta_w, dtype=np.float32)
    beta_b = np.asarray(beta_b, dtype=np.float32)

    ctxb = emb_ptw[ptw_ids[0]] + b0[None, :]        # [P, A] (b0 folded in)

    # w0 with a-columns replicated into the (p2, a64) / (q4, p2, a16) layouts
    w0_rep = np.empty((D, NC_MAIN, 2, 64), np.float32)
    for c in range(NC_MAIN):
        w0_rep[:, c, :, :] = w0[:, 64 * c: 64 * (c + 1)][:, None, :]
    w0_rep = w0_rep.reshape(D, NC_MAIN * 128).astype(NPBF16)
    w0_rem = np.tile(w0[:, 384:400], (1, 8)).astype(NPBF16)      # (q,p2,a)

    # ctx scalars: rows (p2, a64); col (c, ph, pg): ctxb[ph*16+pg*2+p2, 64c+a]
    ctx_sc = np.zeros((128, NC_MAIN * NPH * NPG), np.float32)
    for c in range(NC_MAIN):
        for ph in range(NPH):
            for pg in range(NPG):
                col = (c * NPH + ph) * NPG + pg
                for p2 in range(2):
                    p = ph * 16 + pg * 2 + p2
                    ctx_sc[p2 * 64:(p2 + 1) * 64, col] = ctxb[p, 64 * c: 64 * (c + 1)]
    # rem rows (q4, p2, a16); col (ph, T): p = ph*16 + T*8 + q*2 + p2
    ctx_rem = np.zeros((128, NPH * NT), np.float32)
    for ph in range(NPH):
        for T in range(NT):
            col = ph * NT + T
            for q in range(4):
                for p2 in range(2):
                    p = ph * 16 + T * 8 + q * 2 + p2
                    r0 = q * 32 + p2 * 16
                    ctx_rem[r0:r0 + AREM, col] = ctxb[p, 384:400]

    # block-diagonal beta: rows (p2, a64); col (c, p2', h)
    beta_bd = np.zeros((128, NC_MAIN, 2, 8), np.float32)
    for c in range(NC_MAIN):
        for p2 in range(2):
            beta_bd[p2 * 64:(p2 + 1) * 64, c, p2, :] = beta_w[64 * c: 64 * (c + 1), :]
    beta_bd = beta_bd.reshape(128, NC_MAIN * 16).astype(NPBF16)
    # rem: rows (q, p2, a16); col j = 32*qq + 8*p2' + h (j%32>=16 -> zero col)
    beta_rem = np.zeros((128, 128), np.float32)
    for q in range(4):
        for p2 in range(2):
            r0 = q * 32 + p2 * 16
            beta_rem[r0:r0 + AREM, 32 * q + 8 * p2: 32 * q + 8 * p2 + 8] = \
                beta_w[384:400, :]
    beta_rem = beta_rem.astype(NPBF16)

    # head-sum 0/1 matrix: col (T, j=p_local in unit); rows 32*qq + 8*p2 + h
    hsum = np.zeros((128, NT, 16), np.float32)
    for T in range(NT):
        for j in range(16):
            if j // 8 != T:
                continue
            jj = j - 8 * T
            qq, p2 = jj // 2, jj % 2
            hsum[32 * qq + 8 * p2: 32 * qq + 8 * p2 + 8, T, j] = 1.0
    hsum = hsum.reshape(128, NT * 16).astype(NPBF16)

    ident = np.eye(128, dtype=np.float32)

    exp_bias = np.zeros((128, 1), np.float32)
    for r in range(128):
        if r % 32 < 16:
            exp_bias[r, 0] = beta_b[r % 8]

    return {
        "w0_rep": w0_rep, "w0_rem": w0_rem,
        "ctx_sc": ctx_sc, "ctx_rem": ctx_rem,
        "beta_bd": beta_bd, "beta_rem": beta_rem,
        "hsum": hsum, "ident": ident, "exp_bias": exp_bias,
    }


_NC_CACHE = []
LAST_RESULTS = []


def get_nc():
    if not _NC_CACHE:
        _NC_CACHE.append(build_program())
    return _NC_CACHE[0]


def make_in_maps(omc_idx, ptw_ids, emb_gene, emb_ptw, w0, b0, beta_w, beta_b):
    aux = build_aux(ptw_ids, emb_ptw, w0, b0, beta_w, beta_b)
    emb = np.ascontiguousarray(np.asarray(emb_gene, dtype=np.float32))
    omc = np.asarray(omc_idx).astype(np.int32)
    in_maps = []
    for i in range(NCORES):
        m = dict(aux)
        m["emb_gene"] = emb
        m["idx_loc"] = np.ascontiguousarray(
            omc[BLOC * i: BLOC * (i + 1)].reshape(-1)
        )
        in_maps.append(m)
    return in_maps


def kernel(omc_idx, ptw_ids, emb_gene, emb_ptw, w0, b0, beta_w, beta_b):
    in_maps = make_in_maps(
        omc_idx, ptw_ids, emb_gene, emb_ptw, w0, b0, beta_w, beta_b
    )
    nc = get_nc()
    res = run_bass_kernel_spmd(nc, in_maps, list(range(NCORES)))
    LAST_RESULTS.clear()
    LAST_RESULTS.append(res)
    out = np.concatenate(
        [np.asarray(res.results[i]["out_loc"]) for i in range(NCORES)], axis=0
    )
    return out.astype(np.float32)
